# revision 1
# baseline (speedup 1.0000x reference)
"""Trainium2 Bass kernel for nn_AttentionHead (B=8, T=2048, D=1024, H=64).

Single attention head with additive relative-position scores:
    k = x@Wk + bk; q = x@Wq + bq; v = x@Wv
    S = (q k^T) sqrt(H) + einsum(btc,tvc->btv)(q, rel)  [+ causal mask]
    out = softmax(S) @ v

Distribution: query-block parallel over 8 NeuronCores. Core c owns query
blocks {c, 15-c} (128 rows each) so causal work is balanced. One SPMD
program runs on every core; per-core differences (which rel rows, which
causal mask, which q columns) are carried entirely by the input data.

Numerics: the PE's fp32 matmul path is only ~tf32 accurate, which is not
enough for the large-magnitude logits here (softmax near-ties amplify
score error). All score-path matmuls therefore run as bf16 hi/lo split
products (error ~2^-17); x, W and rel are split on the host. v and the
P@V reduction run in single bf16 (linear error only).
"""

import os
from contextlib import ExitStack

import numpy as np
import ml_dtypes

import concourse.bass as bass
import concourse.tile as tile
from concourse import bacc, mybir
from concourse.bass_utils import run_bass_kernel_spmd

BF16 = mybir.dt.bfloat16
F16 = mybir.dt.float16
F32 = mybir.dt.float32

# problem shape (hardcoded per contract)
B, T, D, H = 8, 2048, 1024, 64
TB = 128              # query-block rows
NBLK = T // TB        # 16
NCORES = 8
NEG = -1.0e9

LAST_EXEC_NS = None


def _cfg(causal: bool):
    # per-core uniform padded extents for the (small, big) block slots
    if causal:
        exts = (1024, 2048)
    else:
        exts = (2048, 2048)
    return {
        "B": B, "T": T, "D": D, "H": H, "TB": TB,
        "exts": exts, "smax": T,
    }


def build_nc(cfg):
    Bc, Tc, Dc, Hc, TBc = cfg["B"], cfg["T"], cfg["D"], cfg["H"], cfg["TB"]
    exts = cfg["exts"]
    smax = cfg["smax"]
    ND = Dc // 128                 # d-tiles
    NQ = 2 * TBc                   # own query rows (2 blocks)
    SCH = 512                      # s-chunk for projections / scores
    NPAIR = TBc // 2               # 64 t-pairs per block
    NGRP = NPAIR // 4              # 16 groups of 4 pairs

    nc = bacc.Bacc("TRN2", target_bir_lowering=False, debug=False,
                   num_devices=NCORES)

    # ---- I/O ----
    xh = nc.dram_tensor("xh", [Dc, Bc, Tc], BF16, kind="ExternalInput")
    xl = nc.dram_tensor("xl", [Dc, Bc, Tc], BF16, kind="ExternalInput")
    xqh = nc.dram_tensor("xqh", [Dc, 2, Bc, TBc], BF16, kind="ExternalInput")
    xql = nc.dram_tensor("xql", [Dc, 2, Bc, TBc], BF16, kind="ExternalInput")
    wkh = nc.dram_tensor("wkh", [Dc, Hc], BF16, kind="ExternalInput")
    wkl = nc.dram_tensor("wkl", [Dc, Hc], BF16, kind="ExternalInput")
    wqh = nc.dram_tensor("wqh", [Dc, Hc], BF16, kind="ExternalInput")
    wql = nc.dram_tensor("wql", [Dc, Hc], BF16, kind="ExternalInput")
    wv = nc.dram_tensor("wv", [Dc, Hc], BF16, kind="ExternalInput")
    bk8 = nc.dram_tensor("bk8", [Hc, 1], F32, kind="ExternalInput")
    bq_ = nc.dram_tensor("bq", [Hc, 1], F32, kind="ExternalInput")
    relh = nc.dram_tensor("relh", [2, TBc, Hc, Tc], BF16, kind="ExternalInput")
    rell = nc.dram_tensor("rell", [2, TBc, Hc, Tc], BF16, kind="ExternalInput")
    maskA = nc.dram_tensor("maskA", [TBc, exts[0]], BF16, kind="ExternalInput")
    maskB = nc.dram_tensor("maskB", [TBc, exts[1]], BF16, kind="ExternalInput")
    identf = nc.dram_tensor("identf", [128, 128], F32, kind="ExternalInput")
    identb = nc.dram_tensor("identb", [128, 128], F16, kind="ExternalInput")
    out = nc.dram_tensor("out", [Bc, 2, TBc, Hc], F32, kind="ExternalOutput")

    NST = smax // 128              # s-tiles for V
    with tile.TileContext(nc) as tc:
        # ---------------- persistent tiles ----------------
        with (
            tc.tile_pool(name="persist", bufs=1) as pp,
            tc.tile_pool(name="weights", bufs=1) as pw,
        ):
            # k stacked: rows 0-63 = kT_hi, rows 64-127 = kT_lo ; cols (b, s)
            kstack = pp.tile([128, Bc * smax], BF16, tag="kstack")
            # q stacks: cols (blk, b, t)
            qmain = pp.tile([128, NQ * Bc], BF16, tag="qmain")   # hi top, lo bottom
            qcorr = pp.tile([128, NQ * Bc], BF16, tag="qcorr")   # lo top, hi bottom
            # V natural: [s-part, (b, stile, h)]
            vnat = pp.tile([128, Bc * NST * Hc], F16, tag="vnat")
            mA = pp.tile([TBc, exts[0]], BF16, tag="maskA")
            mB = pp.tile([TBc, exts[1]], BF16, tag="maskB")
            idf = pw.tile([128, 128], F32, tag="identf")
            idb = pw.tile([128, 128], F16, tag="identb")
            wk_t = pw.tile([128, ND, 2, Hc], BF16, tag="wk")     # (dtile, hi/lo, h)
            wq_t = pw.tile([128, ND, 2, Hc], BF16, tag="wq")
            wv_t = pw.tile([128, ND, Hc], BF16, tag="wv")
            bk_t = pw.tile([Hc, 1], F32, tag="bk")
            bq_t = pw.tile([Hc, 1], F32, tag="bq")

            nc.sync.dma_start(mA, maskA.ap())
            nc.sync.dma_start(mB, maskB.ap())
            nc.sync.dma_start(idf, identf.ap())
            nc.sync.dma_start(idb, identb.ap())
            nc.sync.dma_start(
                wk_t[:, :, 0, :], wkh.ap().rearrange("(n p) h -> p n h", p=128))
            nc.sync.dma_start(
                wk_t[:, :, 1, :], wkl.ap().rearrange("(n p) h -> p n h", p=128))
            nc.sync.dma_start(
                wq_t[:, :, 0, :], wqh.ap().rearrange("(n p) h -> p n h", p=128))
            nc.sync.dma_start(
                wq_t[:, :, 1, :], wql.ap().rearrange("(n p) h -> p n h", p=128))
            nc.sync.dma_start(
                wv_t, wv.ap().rearrange("(n p) h -> p n h", p=128))
            nc.sync.dma_start(bk_t, bk8.ap())
            nc.sync.dma_start(bq_t, bq_.ap())

            # ---------------- streaming phases (interleaved) ----------------
            with ExitStack() as stk:
                ent = stk.enter_context
                pxh = ent(tc.tile_pool(name="xhstream", bufs=2))
                pxl = ent(tc.tile_pool(name="xlstream", bufs=1))
                pst = ent(tc.tile_pool(name="pstage", bufs=2))
                pbd = ent(tc.tile_pool(name="bd", bufs=2))
                prel = ent(tc.tile_pool(name="relstream", bufs=2))
                pstage = ent(tc.tile_pool(name="stage", bufs=1))
                pS = ent(tc.tile_pool(name="Spool", bufs=8))
                pP = ent(tc.tile_pool(name="Ppool", bufs=1))
                pPT = ent(tc.tile_pool(name="PTpool", bufs=2))
                po = ent(tc.tile_pool(name="outpool", bufs=2))
                pstat = ent(tc.tile_pool(name="stats", bufs=4))
                ppmm = ent(tc.tile_pool(name="psmm512", bufs=2, space="PSUM"))
                ppv = ent(tc.tile_pool(name="psv", bufs=1, space="PSUM"))
                ppr = ent(tc.tile_pool(name="psrel", bufs=2, space="PSUM"))
                ppsm = ent(tc.tile_pool(name="pssmall", bufs=2, space="PSUM"))
                pppt = ent(tc.tile_pool(name="pspt", bufs=1, space="PSUM"))
                # ---- q projection over own columns: cols (blk, b, t) ----
                xqf = xqh.ap().rearrange("(n p) k b t -> p n (k b t)", p=128)
                xqlf = xql.ap().rearrange("(n p) k b t -> p n (k b t)", p=128)
                for ci in range(2 * Bc * TBc // SCH):
                    c0 = ci * SCH
                    xht = pxh.tile([128, ND, SCH], BF16, tag="xh")
                    xlt = pxl.tile([128, ND, SCH], BF16, tag="xl")
                    nc.sync.dma_start(xht, xqf[:, :, c0:c0 + SCH])
                    nc.sync.dma_start(xlt, xqlf[:, :, c0:c0 + SCH])
                    psq = ppmm.tile([Hc, SCH], F32, tag="mm512")
                    for dt_ in range(ND):
                        nc.tensor.matmul(psq, wq_t[:, dt_, 0, :], xht[:, dt_],
                                         start=(dt_ == 0), stop=False)
                    for dt_ in range(ND):
                        nc.tensor.matmul(psq, wq_t[:, dt_, 1, :], xht[:, dt_],
                                         start=False, stop=False)
                    for dt_ in range(ND):
                        nc.tensor.matmul(psq, wq_t[:, dt_, 0, :], xlt[:, dt_],
                                         start=False, stop=(dt_ == ND - 1))
                    qtmp = pst.tile([Hc, SCH], F32, tag="ktmp")
                    nc.scalar.activation(qtmp, psq,
                                         mybir.ActivationFunctionType.Identity,
                                         bias=bq_t[:, :], scale=1.0)
                    nc.vector.tensor_copy(qmain[0:Hc, c0:c0 + SCH], qtmp)
                    nc.vector.tensor_tensor(
                        qmain[Hc:128, c0:c0 + SCH], qtmp,
                        qmain[0:Hc, c0:c0 + SCH], mybir.AluOpType.subtract)
                    nc.vector.tensor_copy(qcorr[Hc:128, c0:c0 + SCH],
                                          qmain[0:Hc, c0:c0 + SCH])
                    nc.vector.tensor_copy(qcorr[0:Hc, c0:c0 + SCH],
                                          qmain[Hc:128, c0:c0 + SCH])

                # ---- block-diagonal q tiles for both blocks ----
                bds = []
                for blk in range(2):
                    bdh = pbd.tile([128, NPAIR * 16], BF16, tag="bdh",
                                   name=f"bdh_{blk}")
                    bdl = pbd.tile([128, NPAIR * 16], BF16, tag="bdl",
                                   name=f"bdl_{blk}")
                    nc.vector.memset(bdh, 0.0)
                    nc.vector.memset(bdl, 0.0)
                    qblk_h = (qmain[0:Hc, blk * Bc * TBc:(blk + 1) * Bc * TBc]
                              .rearrange("c (b t) -> c b t", b=Bc))
                    qblk_l = (qmain[Hc:128, blk * Bc * TBc:(blk + 1) * Bc * TBc]
                              .rearrange("c (b t) -> c b t", b=Bc))
                    for j in range(2):
                        dst_h = (bdh[j * Hc:(j + 1) * Hc]
                                 .rearrange("c (p s) -> c p s", s=16)
                                 [:, :, j * 8:j * 8 + 8])
                        src_h = qblk_h[:, :, j::2].rearrange("c b p -> c p b")
                        nc.vector.tensor_copy(dst_h, src_h)
                        dst_l = (bdl[j * Hc:(j + 1) * Hc]
                                 .rearrange("c (p s) -> c p s", s=16)
                                 [:, :, j * 8:j * 8 + 8])
                        src_l = qblk_l[:, :, j::2].rearrange("c b p -> c p b")
                        nc.vector.tensor_copy(dst_l, src_l)
                    bds.append((bdh, bdl))

                # S tiles for both blocks, pre-filled by the rel shuffle
                S_all = []
                for blk in range(2):
                    S_all.append([pS.tile([TBc, exts[blk]], F32, tag="S",
                                          name=f"S_{blk}_{i}")
                                  for i in range(Bc)])

                # ---- generator: k/v projection chunks ----
                def proj_steps():
                    xf = xh.ap().rearrange("(n p) b t -> p n (b t)", p=128)
                    xlf = xl.ap().rearrange("(n p) b t -> p n (b t)", p=128)
                    for ci in range(Bc * smax // SCH):
                        c0 = ci * SCH
                        bidx = c0 // smax
                        s0 = c0 % smax
                        xht = pxh.tile([128, ND, SCH], BF16, tag="xh")
                        xlt = pxl.tile([128, ND, SCH], BF16, tag="xl")
                        nc.sync.dma_start(xht, xf[:, :, c0:c0 + SCH])
                        nc.sync.dma_start(xlt, xlf[:, :, c0:c0 + SCH])
                        psk = ppmm.tile([Hc, SCH], F32, tag="mm512")
                        for dt_ in range(ND):
                            nc.tensor.matmul(psk, wk_t[:, dt_, 0, :],
                                             xht[:, dt_],
                                             start=(dt_ == 0), stop=False)
                        for dt_ in range(ND):
                            nc.tensor.matmul(psk, wk_t[:, dt_, 1, :],
                                             xht[:, dt_],
                                             start=False, stop=False)
                        for dt_ in range(ND):
                            nc.tensor.matmul(psk, wk_t[:, dt_, 0, :],
                                             xlt[:, dt_],
                                             start=False, stop=(dt_ == ND - 1))
                        ktmp = pst.tile([Hc, SCH], F32, tag="ktmp")
                        nc.scalar.activation(
                            ktmp, psk, mybir.ActivationFunctionType.Identity,
                            bias=bk_t[:, :], scale=1.0)
                        nc.vector.tensor_copy(kstack[0:Hc, c0:c0 + SCH], ktmp)
                        nc.vector.tensor_tensor(
                            kstack[Hc:128, c0:c0 + SCH], ktmp,
                            kstack[0:Hc, c0:c0 + SCH],
                            mybir.AluOpType.subtract)
                        psv = ppv.tile([Hc, SCH], F32, tag="pv")
                        for dt_ in range(ND):
                            nc.tensor.matmul(psv, wv_t[:, dt_], xht[:, dt_],
                                             start=(dt_ == 0),
                                             stop=(dt_ == ND - 1))
                        vtmp = pst.tile([Hc, SCH], F32, tag="vtmp")
                        nc.any.tensor_copy(vtmp, psv)
                        for sub in range(SCH // 128):
                            pvt = ppsm.tile([128, Hc], F32, tag="small64")
                            nc.tensor.transpose(
                                pvt, vtmp[:, sub * 128:(sub + 1) * 128],
                                idf[0:Hc, 0:Hc])
                            st = (s0 + sub * 128) // 128
                            nc.any.tensor_copy(
                                vnat[:, bidx * NST * Hc + st * Hc:
                                     bidx * NST * Hc + (st + 1) * Hc], pvt)
                        yield

                # ---- generator: rel-score streaming ----
                def rel_steps():
                    relf = relh.ap()
                    relfl = rell.ap()
                    for blk in range(2):
                        ext = exts[blk]
                        nch = ext // SCH
                        bdh, bdl = bds[blk]
                        for g in range(NGRP):
                            stg = pstage.tile([128, ext], F32, tag="stage")
                            for ch in range(nch):
                                v0 = ch * SCH
                                psr = ppr.tile([128, SCH], F32, tag="pr")
                                rht = prel.tile([128, 4, SCH], BF16, tag="rh")
                                rlt = prel.tile([128, 4, SCH], BF16, tag="rl")
                                # one DMA covers the 4 pairs of the group:
                                # partition (j, c), free (pair u, v)
                                src_h = (relf[blk, 8 * g:8 * g + 8, :,
                                              v0:v0 + SCH]
                                         .rearrange("(u j) c v -> (j c) u v",
                                                    j=2))
                                src_l = (relfl[blk, 8 * g:8 * g + 8, :,
                                               v0:v0 + SCH]
                                         .rearrange("(u j) c v -> (j c) u v",
                                                    j=2))
                                nc.sync.dma_start(rht, src_h)
                                nc.sync.dma_start(rlt, src_l)
                                for u in range(4):
                                    p = 4 * g + u
                                    pslice = psr[32 * u:32 * u + 16, :]
                                    bd_h = bdh[:, p * 16:p * 16 + 16]
                                    bd_l = bdl[:, p * 16:p * 16 + 16]
                                    nc.tensor.matmul(pslice, bd_h, rht[:, u],
                                                     start=True, stop=False,
                                                     tile_position=(0, 32 * u))
                                    nc.tensor.matmul(pslice, bd_l, rht[:, u],
                                                     start=False, stop=False,
                                                     tile_position=(0, 32 * u))
                                    nc.tensor.matmul(pslice, bd_h, rlt[:, u],
                                                     start=False, stop=True,
                                                     tile_position=(0, 32 * u))
                                nc.any.tensor_copy(stg[:, v0:v0 + SCH], psr)
                                yield
                            for j in range(2):
                                for b in range(Bc):
                                    nc.scalar.dma_start(
                                        S_all[blk][b][8 * g + j:8 * g + 8:2, :],
                                        stg[8 * j + b::32, :])

                # ---- drive the two streams interleaved (3 rel : 1 proj) ----
                pgen = proj_steps()
                rgen = rel_steps()
                done_p = done_r = False
                while not (done_p and done_r):
                    if not done_p:
                        done_p = next(pgen, "end") == "end"
                    for _ in range(3):
                        if not done_r:
                            done_r = next(rgen, "end") == "end"

                # ---- per (block, batch): qk scores, softmax, P^T, AV ----
                for blk in range(2):
                    ext = exts[blk]
                    nch = ext // SCH
                    msk = mA if blk == 0 else mB
                    for b in range(Bc):
                        S = S_all[blk][b]
                        qm = qmain[0:Hc, (blk * Bc + b) * TBc:
                                   (blk * Bc + b + 1) * TBc]
                        qc = qcorr[:, (blk * Bc + b) * TBc:
                                   (blk * Bc + b + 1) * TBc]
                        for ch in range(nch):
                            s0 = ch * SCH
                            psS = ppmm.tile([TBc, SCH], F32, tag="mm512")
                            cols = slice(b * smax + s0, b * smax + s0 + SCH)
                            nc.tensor.matmul(psS, qm, kstack[0:Hc, cols],
                                             start=True, stop=False)
                            nc.tensor.matmul(psS, qc, kstack[:, cols],
                                             start=False, stop=True)
                            nc.vector.tensor_tensor(
                                S[:, s0:s0 + SCH], psS, S[:, s0:s0 + SCH],
                                mybir.AluOpType.add)
                            nc.vector.tensor_tensor(
                                S[:, s0:s0 + SCH], S[:, s0:s0 + SCH],
                                msk[:, s0:s0 + SCH], mybir.AluOpType.add)
                        negmax = pstat.tile([TBc, 1], F32, tag="negmax")
                        zsum = pstat.tile([TBc, 1], F32, tag="zsum")
                        rz = pstat.tile([TBc, 1], F32, tag="rz")
                        nc.vector.tensor_reduce(negmax, S,
                                                mybir.AxisListType.X,
                                                mybir.AluOpType.max,
                                                negate=True)
                        P = pP.tile([TBc, ext], F16, tag="P")
                        nc.scalar.activation(P, S,
                                             mybir.ActivationFunctionType.Exp,
                                             bias=negmax[:, :], scale=1.0,
                                             accum_out=zsum[:, :])
                        nc.vector.reciprocal(rz, zsum)
                        pso = ppsm.tile([TBc, Hc], F32, tag="small64")
                        for st in range(ext // 128):
                            ppt = pppt.tile([128, 128], F16, tag="pt")
                            nc.tensor.transpose(
                                ppt, P[:, st * 128:(st + 1) * 128], idb)
                            ptt = pPT.tile([128, 128], F16, tag="ptt")
                            nc.any.tensor_copy(ptt, ppt)
                            nc.tensor.matmul(
                                pso, ptt,
                                vnat[:, (b * NST + st) * Hc:
                                     (b * NST + st + 1) * Hc],
                                start=(st == 0), stop=(st == ext // 128 - 1))
                        osb = po.tile([TBc, Hc], F32, tag="osb")
                        nc.vector.tensor_scalar_mul(osb, pso, rz[:, :])
                        nc.sync.dma_start(out.ap()[b, blk], osb)

    nc.compile()
    return nc


def _split(a):
    hi = np.asarray(a, dtype=np.float32).astype(ml_dtypes.bfloat16)
    lo = (np.asarray(a, dtype=np.float32) - hi.astype(np.float32)).astype(
        ml_dtypes.bfloat16)
    return hi, lo


def kernel(x, Wk, bk, Wq, bq, Wv, rel_pos_emb, mask, **_unused):
    global LAST_EXEC_NS
    x = np.asarray(x, dtype=np.float32)
    Wk = np.asarray(Wk, dtype=np.float32)
    bk = np.asarray(bk, dtype=np.float32)
    Wq = np.asarray(Wq, dtype=np.float32)
    bq = np.asarray(bq, dtype=np.float32)
    Wv = np.asarray(Wv, dtype=np.float32)
    rel = np.asarray(rel_pos_emb, dtype=np.float32)
    causal = bool(np.asarray(mask).item())
    cfg = _cfg(causal)
    exts = cfg["exts"]

    scale = np.float32(np.sqrt(H))
    # xT: [D, B, T]
    xT = np.ascontiguousarray(x.transpose(2, 0, 1))
    xh, xl = _split(xT)
    wkh, wkl = _split(Wk * scale)
    wqh, wql = _split(Wq)
    wvh = Wv.astype(ml_dtypes.bfloat16)
    bk8 = (bk * scale).reshape(H, 1).astype(np.float32)
    bqr = bq.reshape(H, 1).astype(np.float32)
    # relT: [T, H, T] (t, c, v)
    relT = np.ascontiguousarray(rel.transpose(0, 2, 1))
    rth, rtl = _split(relT)
    identf = np.eye(128, dtype=np.float32)
    identb = np.eye(128, dtype=np.float16)

    in_maps = []
    blocks = []
    for c in range(NCORES):
        bA, bB = c, NBLK - 1 - c
        blocks.append((bA, bB))
        relh_c = np.stack([rth[bA * TB:(bA + 1) * TB], rth[bB * TB:(bB + 1) * TB]])
        rell_c = np.stack([rtl[bA * TB:(bA + 1) * TB], rtl[bB * TB:(bB + 1) * TB]])
        xqh_c = np.stack([xh[:, :, bA * TB:(bA + 1) * TB],
                          xh[:, :, bB * TB:(bB + 1) * TB]], axis=1)
        xql_c = np.stack([xl[:, :, bA * TB:(bA + 1) * TB],
                          xl[:, :, bB * TB:(bB + 1) * TB]], axis=1)
        masks = []
        for slot, blkid in ((0, bA), (1, bB)):
            ext = exts[slot]
            t_idx = blkid * TB + np.arange(TB)[:, None]
            s_idx = np.arange(ext)[None, :]
            if causal:
                m = np.where(s_idx <= t_idx, 0.0, NEG)
            else:
                m = np.zeros((TB, ext))
            masks.append(np.ascontiguousarray(m.astype(ml_dtypes.bfloat16)))
        in_maps.append({
            "xh": xh, "xl": xl,
            "xqh": np.ascontiguousarray(xqh_c),
            "xql": np.ascontiguousarray(xql_c),
            "wkh": wkh, "wkl": wkl, "wqh": wqh, "wql": wql, "wv": wvh,
            "bk8": bk8, "bq": bqr,
            "relh": np.ascontiguousarray(relh_c),
            "rell": np.ascontiguousarray(rell_c),
            "maskA": masks[0], "maskB": masks[1],
            "identf": identf, "identb": identb,
        })

    nc = build_nc(cfg)
    if os.environ.get("KERNEL_TRACE") == "1":
        # warm PJRT with a tiny no-op execution is not possible here; the
        # profile hook needs an initialized backend, so trigger init first.
        import jax
        jax.devices()
        try:
            res = run_bass_kernel_spmd(
                nc, in_maps, core_ids=list(range(NCORES)), trace=True)
        except RuntimeError:
            res = run_bass_kernel_spmd(
                nc, in_maps, core_ids=list(range(NCORES)))
    else:
        res = run_bass_kernel_spmd(nc, in_maps, core_ids=list(range(NCORES)))
    LAST_EXEC_NS = res.exec_time_ns

    out = np.empty((B, T, H), dtype=np.float32)
    for c in range(NCORES):
        oc = res.results[c]["out"]          # [B, 2, TB, H]
        bA, bB = blocks[c]
        out[:, bA * TB:(bA + 1) * TB] = oc[:, 0]
        out[:, bB * TB:(bB + 1) * TB] = oc[:, 1]
    return out



# revision 4
# speedup vs baseline: 1.7392x; 1.7392x over previous
"""Trainium2 Bass kernel for nn_AttentionHead (B=8, T=2048, D=1024, H=64).

Single attention head with additive relative-position scores:
    k = x@Wk + bk; q = x@Wq + bq; v = x@Wv
    S = (q k^T) sqrt(H) + einsum(btc,tvc->btv)(q, rel)  [+ causal mask]
    out = softmax(S) @ v

Distribution: query-block parallel over 8 NeuronCores. Core c owns query
blocks {c, 15-c} (128 rows each) so causal work is balanced. One SPMD
program runs on every core; per-core differences (which rel rows, which
causal mask, which q columns) are carried entirely by the input data.

Numerics (validated against the reference data on CPU): x, W and rel are
single-stream fp16 (exact products, fp32 accumulate on the PE). k and q
are computed in fp32 and split hi/lo bf16 on device for the q·k product
(error ~2^-17); the rel product runs single fp16 (error ~1e-3 on logits).
Measured end-to-end rel-err ~1.2e-2 vs the 2e-2 gate.
"""

import os
from contextlib import ExitStack

import numpy as np
import ml_dtypes

import concourse.bass as bass
import concourse.tile as tile
from concourse import bacc, mybir
from concourse.bass_utils import run_bass_kernel_spmd

BF16 = mybir.dt.bfloat16
F16 = mybir.dt.float16
F32 = mybir.dt.float32

# problem shape (hardcoded per contract)
B, T, D, H = 8, 2048, 1024, 64
TB = 128              # query-block rows
NBLK = T // TB        # 16
NCORES = 8
NEG = -1.0e9

LAST_EXEC_NS = None


def _cfg(causal: bool):
    # per-core uniform padded extents for the (small, big) block slots
    if causal:
        exts = (1024, 2048)
    else:
        exts = (2048, 2048)
    return {
        "B": B, "T": T, "D": D, "H": H, "TB": TB,
        "exts": exts, "smax": T,
    }


def build_nc(cfg):
    Bc, Tc, Dc, Hc, TBc = cfg["B"], cfg["T"], cfg["D"], cfg["H"], cfg["TB"]
    exts = cfg["exts"]
    smax = cfg["smax"]
    ND = Dc // 128                 # d-tiles
    SCH = 512                      # s-chunk for projections / scores
    NPAIR = TBc // 2               # 64 t-pairs per block
    NGRP = NPAIR // 4              # 16 groups of 4 pairs
    NKV = Bc * smax // SCH         # 32 kv-projection chunks
    NQC = 2 * Bc * TBc // SCH      # 4 q-projection chunks
    NCHS = [e // SCH for e in exts]
    NRELC = NGRP * sum(NCHS)       # rel stream chunks total

    nc = bacc.Bacc("TRN2", target_bir_lowering=False, debug=False,
                   num_devices=NCORES)

    # ---- I/O (all host-packed in exact DMA stream order) ----
    xs = nc.dram_tensor("xs", [NKV, 128, ND, SCH], F16, kind="ExternalInput")
    xqs = nc.dram_tensor("xqs", [NQC, 128, ND, SCH], F16, kind="ExternalInput")
    wkv = nc.dram_tensor("wkv", [128, ND, 128], F16, kind="ExternalInput")
    wq_ = nc.dram_tensor("wq", [128, ND, Hc], F16, kind="ExternalInput")
    bk8 = nc.dram_tensor("bk8", [Hc, 1], F32, kind="ExternalInput")
    bq_ = nc.dram_tensor("bq", [Hc, 1], F32, kind="ExternalInput")
    rels = nc.dram_tensor("rels", [NRELC, 128, 4, SCH], F16,
                          kind="ExternalInput")
    maskA = nc.dram_tensor("maskA", [TBc, exts[0]], BF16, kind="ExternalInput")
    maskB = nc.dram_tensor("maskB", [TBc, exts[1]], BF16, kind="ExternalInput")
    identb = nc.dram_tensor("identb", [128, 128], F16, kind="ExternalInput")
    out = nc.dram_tensor("out", [Bc, 2, TBc, Hc], F32, kind="ExternalOutput")

    NST = smax // 128              # s-tiles for V
    with tile.TileContext(nc) as tc:
        with (
            tc.tile_pool(name="persist", bufs=1) as pp,
            tc.tile_pool(name="weights", bufs=1) as pw,
            tc.tile_pool(name="S16pool", bufs=8) as pS,
        ):
            # k stacked: rows 0-63 = kT_hi, rows 64-127 = kT_lo ; cols (b, s)
            kstack = pp.tile([128, Bc * smax], BF16, tag="kstack")
            # q stacks: cols (blk, b, t)
            qmain = pp.tile([128, 2 * Bc * TBc], BF16, tag="qmain")
            qcorr = pp.tile([128, 2 * Bc * TBc], BF16, tag="qcorr")
            qf16 = pp.tile([Hc, 2 * Bc * TBc], F16, tag="qf16")
            # V natural: [s-part, (b, stile, h)]
            vnat = pp.tile([128, Bc * NST * Hc], F16, tag="vnat")
            mA = pp.tile([TBc, exts[0]], BF16, tag="maskA")
            mB = pp.tile([TBc, exts[1]], BF16, tag="maskB")
            idb = pw.tile([128, 128], F16, tag="identb")
            wkv_t = pw.tile([128, ND, 128], F16, tag="wkv")
            wq_t = pw.tile([128, ND, Hc], F16, tag="wq")
            bk_t = pw.tile([Hc, 1], F32, tag="bk")
            bq_t = pw.tile([Hc, 1], F32, tag="bq")

            nc.sync.dma_start(mA, maskA.ap())
            nc.sync.dma_start(mB, maskB.ap())
            nc.sync.dma_start(idb, identb.ap())
            nc.sync.dma_start(wkv_t, wkv.ap())
            nc.sync.dma_start(wq_t, wq_.ap())
            nc.sync.dma_start(bk_t, bk8.ap())
            nc.sync.dma_start(bq_t, bq_.ap())

            # rel-score tiles, f16, pre-filled by the rel shuffle.
            # 16 tiles share 8 slots: blk1 tiles recycle blk0's slots
            # after the blk0 tails consume them.
            S16 = []
            for blk in range(2):
                S16.append([pS.tile([TBc, exts[blk]], F16, tag="S",
                                    name=f"S_{blk}_{i}")
                            for i in range(Bc)])

            with ExitStack() as stk:
                ent = stk.enter_context
                px = ent(tc.tile_pool(name="xstream", bufs=3))
                pst = ent(tc.tile_pool(name="pstage", bufs=3))
                pbd = ent(tc.tile_pool(name="bd", bufs=2))
                prel = ent(tc.tile_pool(name="relstream", bufs=3))
                pstage = ent(tc.tile_pool(name="stage", bufs=2))
                pSf = ent(tc.tile_pool(name="Sfull", bufs=3))
                pP = ent(tc.tile_pool(name="Ppool", bufs=2))
                pPT = ent(tc.tile_pool(name="PTpool", bufs=2))
                po = ent(tc.tile_pool(name="outpool", bufs=2))
                pstat = ent(tc.tile_pool(name="stats", bufs=4))
                ppmm = ent(tc.tile_pool(name="psmm512", bufs=2, space="PSUM"))
                ppr = ent(tc.tile_pool(name="psrel", bufs=2, space="PSUM"))
                ppt = ent(tc.tile_pool(name="pstrans", bufs=2, space="PSUM"))
                ppo = ent(tc.tile_pool(name="psout", bufs=2, space="PSUM"))

                # ---- q projection over own columns: cols (blk, b, t) ----
                for ci in range(NQC):
                    c0 = ci * SCH
                    xqt = px.tile([128, ND, SCH], F16, tag="xs")
                    nc.sync.dma_start(xqt, xqs.ap()[ci])
                    psq = ppmm.tile([128, SCH], F32, tag="mm512")
                    for dt_ in range(ND):
                        nc.tensor.matmul(psq[0:Hc, :], wq_t[:, dt_],
                                         xqt[:, dt_],
                                         start=(dt_ == 0), stop=(dt_ == ND - 1))
                    qtmp = pst.tile([Hc, SCH], F32, tag="ktmp")
                    nc.scalar.activation(qtmp, psq[0:Hc, :],
                                         mybir.ActivationFunctionType.Identity,
                                         bias=bq_t[:, :], scale=1.0)
                    cols = slice(c0, c0 + SCH)
                    nc.vector.tensor_copy(qf16[:, cols], qtmp)
                    nc.vector.tensor_copy(qmain[0:Hc, cols], qtmp)
                    nc.vector.tensor_tensor(
                        qmain[Hc:128, cols], qtmp,
                        qmain[0:Hc, cols], mybir.AluOpType.subtract)
                    nc.vector.tensor_copy(qcorr[Hc:128, cols],
                                          qmain[0:Hc, cols])
                    nc.vector.tensor_copy(qcorr[0:Hc, cols],
                                          qmain[Hc:128, cols])

                # ---- block-diagonal fp16 q tiles for both blocks ----
                bds = []
                for blk in range(2):
                    bd = pbd.tile([128, NPAIR * 16], F16, tag="bd",
                                  name=f"bd_{blk}")
                    nc.vector.memset(bd, 0.0)
                    qblk = (qf16[:, blk * Bc * TBc:(blk + 1) * Bc * TBc]
                            .rearrange("c (b t) -> c b t", b=Bc))
                    for j in range(2):
                        dst = (bd[j * Hc:(j + 1) * Hc]
                               .rearrange("c (p s) -> c p s", s=16)
                               [:, :, j * 8:j * 8 + 8])
                        src = qblk[:, :, j::2].rearrange("c b p -> c p b")
                        nc.vector.tensor_copy(dst, src)
                    bds.append(bd)

                # ---- generator: fused k/v projection chunks ----
                def kv_steps():
                    for ci in range(NKV):
                        c0 = ci * SCH
                        bidx = c0 // smax
                        s0 = c0 % smax
                        xt = px.tile([128, ND, SCH], F16, tag="xs")
                        nc.sync.dma_start(xt, xs.ap()[ci])
                        pskv = ppmm.tile([128, SCH], F32, tag="mm512")
                        for dt_ in range(ND):
                            nc.tensor.matmul(pskv, wkv_t[:, dt_], xt[:, dt_],
                                             start=(dt_ == 0),
                                             stop=(dt_ == ND - 1))
                        ktmp = pst.tile([Hc, SCH], F32, tag="ktmp")
                        nc.scalar.activation(
                            ktmp, pskv[0:Hc, :],
                            mybir.ActivationFunctionType.Identity,
                            bias=bk_t[:, :], scale=1.0)
                        cols = slice(c0, c0 + SCH)
                        nc.vector.tensor_copy(kstack[0:Hc, cols], ktmp)
                        nc.vector.tensor_tensor(
                            kstack[Hc:128, cols], ktmp,
                            kstack[0:Hc, cols], mybir.AluOpType.subtract)
                        # evacuate the v half on ACT as well: a DVE read of
                        # the same PSUM bank can run concurrently with the
                        # ACT read above — same-bank dual access is fatal.
                        vtmp = pst.tile([Hc, SCH], F16, tag="vtmp")
                        nc.scalar.activation(
                            vtmp, pskv[Hc:128, :],
                            mybir.ActivationFunctionType.Identity, scale=1.0)
                        for sub in range(SCH // 128):
                            pvt = ppt.tile([128, 128], F16, tag="ptrans")
                            nc.tensor.transpose(
                                pvt[:, 0:Hc], vtmp[:, sub * 128:(sub + 1) * 128],
                                idb[0:Hc, 0:Hc])
                            st = (s0 + sub * 128) // 128
                            nc.vector.tensor_copy(
                                vnat[:, bidx * NST * Hc + st * Hc:
                                     bidx * NST * Hc + (st + 1) * Hc],
                                pvt[:, 0:Hc])
                        yield

                # ---- generator: rel-score streaming for one block ----
                def rel_steps(blk):
                    ext = exts[blk]
                    nch = ext // SCH
                    bd = bds[blk]
                    base = 0 if blk == 0 else NGRP * NCHS[0]
                    for g in range(NGRP):
                        stg = pstage.tile([128, exts[1]], F16, tag="stage")
                        for ch in range(nch):
                            v0 = ch * SCH
                            ci = base + g * nch + ch
                            psr = ppr.tile([128, SCH], F32, tag="pr")
                            rht = prel.tile([128, 4, SCH], F16, tag="rh")
                            nc.sync.dma_start(rht, rels.ap()[ci])
                            for u in range(4):
                                p = 4 * g + u
                                nc.tensor.matmul(
                                    psr[32 * u:32 * u + 16, :],
                                    bd[:, p * 16:p * 16 + 16], rht[:, u],
                                    start=True, stop=True,
                                    tile_position=(0, 32 * u))
                            nc.vector.tensor_copy(stg[:, v0:v0 + SCH], psr)
                            yield
                        for j in range(2):
                            for b in range(Bc):
                                nc.scalar.dma_start(
                                    S16[blk][b][8 * g + j:8 * g + 8:2, :],
                                    stg[8 * j + b::32, 0:ext])

                # ---- generator: per-(block,batch) tail ----
                def tail_steps(blk):
                    ext = exts[blk]
                    nch = ext // SCH
                    msk = mA if blk == 0 else mB
                    for b in range(Bc):
                        S = pSf.tile([TBc, exts[1]], F32, tag="Sfull")
                        qm = qmain[0:Hc, (blk * Bc + b) * TBc:
                                   (blk * Bc + b + 1) * TBc]
                        qc = qcorr[:, (blk * Bc + b) * TBc:
                                   (blk * Bc + b + 1) * TBc]
                        for ch in range(nch):
                            s0 = ch * SCH
                            psS = ppmm.tile([128, SCH], F32, tag="mm512")
                            cols = slice(b * smax + s0, b * smax + s0 + SCH)
                            nc.tensor.matmul(psS[0:TBc, :], qm,
                                             kstack[0:Hc, cols],
                                             start=True, stop=False)
                            nc.tensor.matmul(psS[0:TBc, :], qc, kstack[:, cols],
                                             start=False, stop=True)
                            nc.vector.tensor_tensor(
                                S[:, s0:s0 + SCH], psS[0:TBc, :],
                                S16[blk][b][:, s0:s0 + SCH],
                                mybir.AluOpType.add)
                            nc.vector.tensor_tensor(
                                S[:, s0:s0 + SCH], S[:, s0:s0 + SCH],
                                msk[:, s0:s0 + SCH], mybir.AluOpType.add)
                        negmax = pstat.tile([TBc, 1], F32, tag="negmax")
                        zsum = pstat.tile([TBc, 1], F32, tag="zsum")
                        rz = pstat.tile([TBc, 1], F32, tag="rz")
                        nc.vector.tensor_reduce(negmax, S[:, 0:ext],
                                                mybir.AxisListType.X,
                                                mybir.AluOpType.max,
                                                negate=True)
                        P = pP.tile([TBc, exts[1]], F16, tag="P")
                        nc.scalar.activation(P[:, 0:ext], S[:, 0:ext],
                                             mybir.ActivationFunctionType.Exp,
                                             bias=negmax[:, :], scale=1.0,
                                             accum_out=zsum[:, :])
                        nc.vector.reciprocal(rz, zsum)
                        pso = ppo.tile([TBc, Hc], F32, tag="pso")
                        for st in range(ext // 128):
                            ptp = ppt.tile([128, 128], F16, tag="ptrans")
                            nc.tensor.transpose(
                                ptp, P[:, st * 128:(st + 1) * 128], idb)
                            ptt = pPT.tile([128, 128], F16, tag="ptt")
                            nc.vector.tensor_copy(ptt, ptp)
                            nc.tensor.matmul(
                                pso, ptt,
                                vnat[:, (b * NST + st) * Hc:
                                     (b * NST + st + 1) * Hc],
                                start=(st == 0), stop=(st == ext // 128 - 1))
                        osb = po.tile([TBc, Hc], F32, tag="osb")
                        nc.vector.tensor_scalar_mul(osb, pso, rz[:, :])
                        nc.sync.dma_start(out.ap()[b, blk], osb)
                        yield

                # ---- drive the streams ----
                # phase A: kv chunks with rel blk0, 2 kv : 3 rel
                kv = kv_steps()
                r0 = rel_steps(0)
                done_kv = done_r0 = False
                while not (done_kv and done_r0):
                    for _ in range(2):
                        if not done_kv:
                            done_kv = next(kv, "end") == "end"
                    for _ in range(3):
                        if not done_r0:
                            done_r0 = next(r0, "end") == "end"
                # phase B: rel blk1 interleaved with blk0 tails
                r1 = rel_steps(1)
                t0 = tail_steps(0)
                done_r1 = done_t0 = False
                while not (done_r1 and done_t0):
                    for _ in range(8):
                        if not done_r1:
                            done_r1 = next(r1, "end") == "end"
                    if not done_t0:
                        done_t0 = next(t0, "end") == "end"
                # phase C: blk1 tails
                for _ in tail_steps(1):
                    pass

    nc.compile()
    return nc


def kernel(x, Wk, bk, Wq, bq, Wv, rel_pos_emb, mask, **_unused):
    global LAST_EXEC_NS
    F16N = ml_dtypes.float16 if hasattr(ml_dtypes, "float16") else np.float16
    x = np.asarray(x, dtype=np.float32)
    Wk = np.asarray(Wk, dtype=np.float32)
    bk = np.asarray(bk, dtype=np.float32)
    Wq = np.asarray(Wq, dtype=np.float32)
    bq = np.asarray(bq, dtype=np.float32)
    Wv = np.asarray(Wv, dtype=np.float32)
    rel = np.asarray(rel_pos_emb, dtype=np.float32)
    causal = bool(np.asarray(mask).item())
    cfg = _cfg(causal)
    exts = cfg["exts"]
    NCHS = [e // 512 for e in exts]
    NGRP = 16

    scale = np.float32(np.sqrt(H))
    # x stream: [D,B,T] -> [32, 128, 8, 512] (chunk, p, n, col), col=(b,t)
    xT = np.ascontiguousarray(x.transpose(2, 0, 1)).astype(np.float16)
    xflat = xT.reshape(8, 128, B * T)
    xs = np.ascontiguousarray(
        xflat.reshape(8, 128, B * T // 512, 512).transpose(2, 1, 0, 3))

    wkv = np.empty((1024, 128), np.float32)
    wkv[:, 0:64] = Wk * scale
    wkv[:, 64:128] = Wv
    wkv = np.ascontiguousarray(
        wkv.reshape(8, 128, 128).transpose(1, 0, 2)).astype(np.float16)
    wq = np.ascontiguousarray(
        Wq.reshape(8, 128, 64).transpose(1, 0, 2)).astype(np.float16)
    bk8 = (bk * scale).reshape(H, 1).astype(np.float32)
    bqr = bq.reshape(H, 1).astype(np.float32)
    identb = np.eye(128, dtype=np.float16)

    # relT: [T, H, T] (t, c, v) in fp16
    relT = np.ascontiguousarray(rel.transpose(0, 2, 1)).astype(np.float16)

    def pack_rel_block(blkid, ext):
        # [TB, H, ext] -> [NGRP*nch, 128, 4, 512]
        nch = ext // 512
        a = relT[blkid * TB:(blkid + 1) * TB, :, 0:ext]
        a = a.reshape(NGRP, 4, 2, H, nch, 512)        # t=(g,u,j), c, (ch,v)
        a = a.transpose(0, 4, 2, 3, 1, 5)             # g, ch, j, c, u, v
        return a.reshape(NGRP * nch, 128, 4, 512)

    in_maps = []
    blocks = []
    for c in range(NCORES):
        bA, bB = c, NBLK - 1 - c
        blocks.append((bA, bB))
        rel_c = np.ascontiguousarray(np.concatenate(
            [pack_rel_block(bA, exts[0]), pack_rel_block(bB, exts[1])]))
        # xq stream: own columns, cols=(blk, b, t)
        xq = np.empty((8, 128, 2, B, TB), np.float16)
        xq[:, :, 0] = xflat.reshape(8, 128, B, T)[:, :, :, bA * TB:(bA + 1) * TB]
        xq[:, :, 1] = xflat.reshape(8, 128, B, T)[:, :, :, bB * TB:(bB + 1) * TB]
        xqs = np.ascontiguousarray(
            xq.reshape(8, 128, 2 * B * TB // 512, 512).transpose(2, 1, 0, 3))
        masks = []
        for slot, blkid in ((0, bA), (1, bB)):
            ext = exts[slot]
            t_idx = blkid * TB + np.arange(TB)[:, None]
            s_idx = np.arange(ext)[None, :]
            if causal:
                m = np.where(s_idx <= t_idx, 0.0, NEG)
            else:
                m = np.zeros((TB, ext))
            masks.append(np.ascontiguousarray(m.astype(ml_dtypes.bfloat16)))
        in_maps.append({
            "xs": xs, "xqs": xqs,
            "wkv": wkv, "wq": wq, "bk8": bk8, "bq": bqr,
            "rels": rel_c,
            "maskA": masks[0], "maskB": masks[1],
            "identb": identb,
        })

    nc = build_nc(cfg)
    if os.environ.get("KERNEL_TRACE") == "1":
        import jax
        jax.devices()
        try:
            res = run_bass_kernel_spmd(
                nc, in_maps, core_ids=list(range(NCORES)), trace=True)
        except (RuntimeError, ModuleNotFoundError):
            res = run_bass_kernel_spmd(
                nc, in_maps, core_ids=list(range(NCORES)))
    else:
        res = run_bass_kernel_spmd(nc, in_maps, core_ids=list(range(NCORES)))
    LAST_EXEC_NS = res.exec_time_ns

    out = np.empty((B, T, H), dtype=np.float32)
    for c in range(NCORES):
        oc = res.results[c]["out"]          # [B, 2, TB, H]
        bA, bB = blocks[c]
        out[:, bA * TB:(bA + 1) * TB] = oc[:, 0]
        out[:, bB * TB:(bB + 1) * TB] = oc[:, 1]
    return out


# revision 9
# speedup vs baseline: 1.9623x; 1.1283x over previous
"""Trainium2 Bass kernel for nn_AttentionHead (B=8, T=2048, D=1024, H=64).

Single attention head with additive relative-position scores:
    k = x@Wk + bk; q = x@Wq + bq; v = x@Wv
    S = (q k^T) sqrt(H) + einsum(btc,tvc->btv)(q, rel)  [+ causal mask]
    out = softmax(S) @ v

Distribution: query-block parallel over 8 NeuronCores. Core c owns query
blocks {c, 15-c} (128 rows each) so causal work is balanced. One SPMD
program runs on every core; per-core differences (which rel rows, which
causal mask, which q columns) are carried entirely by the input data.

Numerics (validated against the reference data on CPU): x, W and rel are
single-stream fp16 (exact products, fp32 accumulate on the PE). k and q
are computed in fp32 and split hi/lo bf16 on device for the q·k product
(error ~2^-17); the rel product runs single fp16 (error ~1e-3 on logits).
Measured end-to-end rel-err ~1.2e-2 vs the 2e-2 gate.
"""

import os
from contextlib import ExitStack

import numpy as np
import ml_dtypes

import concourse.bass as bass
import concourse.tile as tile
from concourse import bacc, mybir
from concourse.bass_utils import run_bass_kernel_spmd

BF16 = mybir.dt.bfloat16
F16 = mybir.dt.float16
F32 = mybir.dt.float32

# problem shape (hardcoded per contract)
B, T, D, H = 8, 2048, 1024, 64
TB = 128              # query-block rows
NBLK = T // TB        # 16
NCORES = 8
NEG = -1.0e9

LAST_EXEC_NS = None


def _cfg(causal: bool):
    # per-core uniform padded extents for the (small, big) block slots
    if causal:
        exts = (1024, 2048)
    else:
        exts = (2048, 2048)
    return {
        "B": B, "T": T, "D": D, "H": H, "TB": TB,
        "exts": exts, "smax": T,
    }


def build_nc(cfg):
    Bc, Tc, Dc, Hc, TBc = cfg["B"], cfg["T"], cfg["D"], cfg["H"], cfg["TB"]
    exts = cfg["exts"]
    smax = cfg["smax"]
    ND = Dc // 128                 # d-tiles
    SCH = 512                      # s-chunk for projections / scores
    NPAIR = TBc // 2               # 64 t-pairs per block
    NGRP = NPAIR // 4              # 16 groups of 4 pairs
    NKV = Bc * smax // SCH         # 32 kv-projection chunks
    NQC = 2 * Bc * TBc // SCH      # 4 q-projection chunks
    NCHS = [e // SCH for e in exts]
    NRELC = NGRP * sum(NCHS)       # rel stream chunks total

    nc = bacc.Bacc("TRN2", target_bir_lowering=False, debug=False,
                   num_devices=NCORES)

    # ---- I/O (all host-packed in exact DMA stream order) ----
    xs = nc.dram_tensor("xs", [NKV, 128, ND, SCH], F16, kind="ExternalInput")
    xqs = nc.dram_tensor("xqs", [NQC, 128, ND, SCH], F16, kind="ExternalInput")
    wkv = nc.dram_tensor("wkv", [128, ND, 128], F16, kind="ExternalInput")
    wq_ = nc.dram_tensor("wq", [128, ND, Hc], F16, kind="ExternalInput")
    bk8 = nc.dram_tensor("bk8", [Hc, 1], F32, kind="ExternalInput")
    bq_ = nc.dram_tensor("bq", [Hc, 1], F32, kind="ExternalInput")
    rels = nc.dram_tensor("rels", [NRELC, 128, 4, SCH], F16,
                          kind="ExternalInput")
    maskA = nc.dram_tensor("maskA", [TBc, exts[0]], BF16, kind="ExternalInput")
    maskB = nc.dram_tensor("maskB", [TBc, exts[1]], BF16, kind="ExternalInput")
    identb = nc.dram_tensor("identb", [128, 128], F16, kind="ExternalInput")
    out = nc.dram_tensor("out", [Bc, 2, TBc, Hc], F32, kind="ExternalOutput")

    NST = smax // 128              # s-tiles for V
    with tile.TileContext(nc) as tc:
        with (
            tc.tile_pool(name="persist", bufs=1) as pp,
            tc.tile_pool(name="weights", bufs=1) as pw,
            tc.tile_pool(name="S16pool", bufs=8) as pS,
        ):
            # k stacked: rows 0-63 = kT_hi, rows 64-127 = kT_lo ; cols (b, s)
            kstack = pp.tile([128, Bc * smax], BF16, tag="kstack")
            # q stacks: cols (blk, b, t)
            qmain = pp.tile([128, 2 * Bc * TBc], BF16, tag="qmain")
            qcorr = pp.tile([128, 2 * Bc * TBc], BF16, tag="qcorr")
            qf16 = pp.tile([Hc, 2 * Bc * TBc], F16, tag="qf16")
            # V natural: [s-part, (b, stile, h)]
            vnat = pp.tile([128, Bc * NST * Hc], F16, tag="vnat")
            mA = pp.tile([TBc, exts[0]], BF16, tag="maskA")
            mB = pp.tile([TBc, exts[1]], BF16, tag="maskB")
            idb = pw.tile([128, 128], F16, tag="identb")
            wkv_t = pw.tile([128, ND, 128], F16, tag="wkv")
            wq_t = pw.tile([128, ND, Hc], F16, tag="wq")
            bk_t = pw.tile([Hc, 1], F32, tag="bk")
            bq_t = pw.tile([Hc, 1], F32, tag="bq")

            nc.sync.dma_start(mA, maskA.ap())
            nc.sync.dma_start(mB, maskB.ap())
            nc.sync.dma_start(idb, identb.ap())
            nc.sync.dma_start(wkv_t, wkv.ap())
            nc.sync.dma_start(wq_t, wq_.ap())
            nc.sync.dma_start(bk_t, bk8.ap())
            nc.sync.dma_start(bq_t, bq_.ap())

            # rel-score tiles, f16, pre-filled by the rel shuffle.
            # One live tile per (block, batch): a shared-slot scheme makes
            # the second block's shuffles wait on the first block's tails.
            S16 = []
            for blk in range(2):
                S16.append([pS.tile([TBc, exts[blk]], F16, tag=f"S{blk}",
                                    name=f"S_{blk}_{i}")
                            for i in range(Bc)])

            with ExitStack() as stk:
                ent = stk.enter_context
                px = ent(tc.tile_pool(name="xstream", bufs=2))
                pst = ent(tc.tile_pool(name="pstage", bufs=2))
                pbd = ent(tc.tile_pool(name="bd", bufs=2))
                prel = ent(tc.tile_pool(name="relstream", bufs=3))
                pstage = ent(tc.tile_pool(name="stage", bufs=2))
                pSf = ent(tc.tile_pool(name="Sfull", bufs=2))
                pP = ent(tc.tile_pool(name="Ppool", bufs=2))
                pPT = ent(tc.tile_pool(name="PTpool", bufs=2))
                po = ent(tc.tile_pool(name="outpool", bufs=2))
                pstat = ent(tc.tile_pool(name="stats", bufs=4))
                ppmm = ent(tc.tile_pool(name="psmm512", bufs=2, space="PSUM"))
                ppr = ent(tc.tile_pool(name="psrel", bufs=2, space="PSUM"))
                ppt = ent(tc.tile_pool(name="pstrans", bufs=2, space="PSUM"))
                ppo = ent(tc.tile_pool(name="psout", bufs=2, space="PSUM"))

                # ---- q projection over own columns: cols (blk, b, t) ----
                for ci in range(NQC):
                    c0 = ci * SCH
                    xqt = px.tile([128, ND, SCH], F16, tag="xs")
                    nc.sync.dma_start(xqt, xqs.ap()[ci])
                    psq = ppmm.tile([128, SCH], F32, tag="mm512")
                    for dt_ in range(ND):
                        nc.tensor.matmul(psq[0:Hc, :], wq_t[:, dt_],
                                         xqt[:, dt_],
                                         start=(dt_ == 0), stop=(dt_ == ND - 1))
                    qtmp = pst.tile([Hc, SCH], F32, tag="ktmp")
                    nc.scalar.activation(qtmp, psq[0:Hc, :],
                                         mybir.ActivationFunctionType.Identity,
                                         bias=bq_t[:, :], scale=1.0)
                    cols = slice(c0, c0 + SCH)
                    nc.vector.tensor_copy(qf16[:, cols], qtmp)
                    nc.vector.tensor_copy(qmain[0:Hc, cols], qtmp)
                    nc.vector.tensor_tensor(
                        qmain[Hc:128, cols], qtmp,
                        qmain[0:Hc, cols], mybir.AluOpType.subtract)
                    nc.vector.tensor_copy(qcorr[Hc:128, cols],
                                          qmain[0:Hc, cols])
                    nc.vector.tensor_copy(qcorr[0:Hc, cols],
                                          qmain[Hc:128, cols])

                # ---- block-diagonal fp16 q tiles for both blocks ----
                bds = []
                for blk in range(2):
                    bd = pbd.tile([128, NPAIR * 16], F16, tag="bd",
                                  name=f"bd_{blk}")
                    nc.vector.memset(bd, 0.0)
                    qblk = (qf16[:, blk * Bc * TBc:(blk + 1) * Bc * TBc]
                            .rearrange("c (b t) -> c b t", b=Bc))
                    for j in range(2):
                        dst = (bd[j * Hc:(j + 1) * Hc]
                               .rearrange("c (p s) -> c p s", s=16)
                               [:, :, j * 8:j * 8 + 8])
                        src = qblk[:, :, j::2].rearrange("c b p -> c p b")
                        nc.vector.tensor_copy(dst, src)
                    bds.append(bd)

                # ---- generator: fused k/v projection chunks ----
                def kv_steps():
                    for ci in range(NKV):
                        c0 = ci * SCH
                        bidx = c0 // smax
                        s0 = c0 % smax
                        xt = px.tile([128, ND, SCH], F16, tag="xs")
                        nc.sync.dma_start(xt, xs.ap()[ci])
                        pskv = ppmm.tile([128, SCH], F32, tag="mm512")
                        for dt_ in range(ND):
                            nc.tensor.matmul(pskv, wkv_t[:, dt_], xt[:, dt_],
                                             start=(dt_ == 0),
                                             stop=(dt_ == ND - 1))
                        ktmp = pst.tile([Hc, SCH], F32, tag="ktmp")
                        nc.scalar.activation(
                            ktmp, pskv[0:Hc, :],
                            mybir.ActivationFunctionType.Identity,
                            bias=bk_t[:, :], scale=1.0)
                        cols = slice(c0, c0 + SCH)
                        nc.vector.tensor_copy(kstack[0:Hc, cols], ktmp)
                        nc.vector.tensor_tensor(
                            kstack[Hc:128, cols], ktmp,
                            kstack[0:Hc, cols], mybir.AluOpType.subtract)
                        # evacuate the v half on ACT as well: a DVE read of
                        # the same PSUM bank can run concurrently with the
                        # ACT read above — same-bank dual access is fatal.
                        vtmp = pst.tile([Hc, SCH], F16, tag="vtmp")
                        nc.scalar.activation(
                            vtmp, pskv[Hc:128, :],
                            mybir.ActivationFunctionType.Identity, scale=1.0)
                        for sub in range(SCH // 128):
                            pvt = ppt.tile([128, 128], F16, tag="ptrans")
                            nc.tensor.transpose(
                                pvt[:, 0:Hc], vtmp[:, sub * 128:(sub + 1) * 128],
                                idb[0:Hc, 0:Hc])
                            st = (s0 + sub * 128) // 128
                            nc.vector.tensor_copy(
                                vnat[:, bidx * NST * Hc + st * Hc:
                                     bidx * NST * Hc + (st + 1) * Hc],
                                pvt[:, 0:Hc])
                        yield

                # ---- generator: rel-score streaming for one block ----
                def rel_steps(blk):
                    ext = exts[blk]
                    nch = ext // SCH
                    bd = bds[blk]
                    base = 0 if blk == 0 else NGRP * NCHS[0]
                    for g in range(NGRP):
                        stg = pstage.tile([128, exts[1]], F16, tag="stage")
                        for ch in range(nch):
                            v0 = ch * SCH
                            ci = base + g * nch + ch
                            psr = ppr.tile([128, SCH], F32, tag="pr")
                            rht = prel.tile([128, 4, SCH], F16, tag="rh")
                            nc.sync.dma_start(rht, rels.ap()[ci])
                            for u in range(4):
                                p = 4 * g + u
                                nc.tensor.matmul(
                                    psr[32 * u:32 * u + 16, :],
                                    bd[:, p * 16:p * 16 + 16], rht[:, u],
                                    start=True, stop=True,
                                    tile_position=(0, 32 * u))
                            nc.vector.tensor_copy(stg[:, v0:v0 + SCH], psr)
                            yield
                        for j in range(2):
                            for b in range(Bc):
                                eng = nc.scalar if (b % 2 == 0) else nc.sync
                                eng.dma_start(
                                    S16[blk][b][8 * g + j:8 * g + 8:2, :],
                                    stg[8 * j + b::32, 0:ext])

                # ---- generator: per-(block,batch) tail ----
                def tail_steps(blk):
                    ext = exts[blk]
                    nch = ext // SCH
                    msk = mA if blk == 0 else mB
                    for b in range(Bc):
                        S = pSf.tile([TBc, exts[1]], F32, tag="Sfull")
                        qm = qmain[0:Hc, (blk * Bc + b) * TBc:
                                   (blk * Bc + b + 1) * TBc]
                        qc = qcorr[:, (blk * Bc + b) * TBc:
                                   (blk * Bc + b + 1) * TBc]
                        for ch in range(nch):
                            s0 = ch * SCH
                            psS = ppmm.tile([128, SCH], F32, tag="mm512")
                            cols = slice(b * smax + s0, b * smax + s0 + SCH)
                            nc.tensor.matmul(psS[0:TBc, :], qm,
                                             kstack[0:Hc, cols],
                                             start=True, stop=False)
                            nc.tensor.matmul(psS[0:TBc, :], qc, kstack[:, cols],
                                             start=False, stop=True)
                            nc.vector.tensor_tensor(
                                S[:, s0:s0 + SCH], psS[0:TBc, :],
                                S16[blk][b][:, s0:s0 + SCH],
                                mybir.AluOpType.add)
                            nc.vector.tensor_tensor(
                                S[:, s0:s0 + SCH], S[:, s0:s0 + SCH],
                                msk[:, s0:s0 + SCH], mybir.AluOpType.add)
                        negmax = pstat.tile([TBc, 1], F32, tag="negmax")
                        zsum = pstat.tile([TBc, 1], F32, tag="zsum")
                        rz = pstat.tile([TBc, 1], F32, tag="rz")
                        nc.vector.tensor_reduce(negmax, S[:, 0:ext],
                                                mybir.AxisListType.X,
                                                mybir.AluOpType.max,
                                                negate=True)
                        P = pP.tile([TBc, exts[1]], F16, tag="P")
                        nc.scalar.activation(P[:, 0:ext], S[:, 0:ext],
                                             mybir.ActivationFunctionType.Exp,
                                             bias=negmax[:, :], scale=1.0,
                                             accum_out=zsum[:, :])
                        nc.vector.reciprocal(rz, zsum)
                        pso = ppo.tile([TBc, Hc], F32, tag="pso")
                        for st in range(ext // 128):
                            ptp = ppt.tile([128, 128], F16, tag="ptrans")
                            nc.tensor.transpose(
                                ptp, P[:, st * 128:(st + 1) * 128], idb)
                            ptt = pPT.tile([128, 128], F16, tag="ptt")
                            nc.vector.tensor_copy(ptt, ptp)
                            nc.tensor.matmul(
                                pso, ptt,
                                vnat[:, (b * NST + st) * Hc:
                                     (b * NST + st + 1) * Hc],
                                start=(st == 0), stop=(st == ext // 128 - 1))
                        osb = po.tile([TBc, Hc], F32, tag="osb")
                        nc.vector.tensor_scalar_mul(osb, pso, rz[:, :])
                        nc.sync.dma_start(out.ap()[b, blk], osb)
                        yield

                # ---- drive the streams ----
                # phase A: kv chunks with rel blk1 (the big block), 1 kv : 2 rel
                kv = kv_steps()
                r1 = rel_steps(1)
                done_kv = done_r1 = False
                while not (done_kv and done_r1):
                    if not done_kv:
                        done_kv = next(kv, "end") == "end"
                    for _ in range(2):
                        if not done_r1:
                            done_r1 = next(r1, "end") == "end"
                # phase B: rel blk0 interleaved with blk1 tails
                r0 = rel_steps(0)
                t1 = tail_steps(1)
                done_r0 = done_t1 = False
                while not (done_r0 and done_t1):
                    for _ in range(4):
                        if not done_r0:
                            done_r0 = next(r0, "end") == "end"
                    if not done_t1:
                        done_t1 = next(t1, "end") == "end"
                # phase C: blk0 tails
                for _ in tail_steps(0):
                    pass

    nc.compile()
    return nc


def kernel(x, Wk, bk, Wq, bq, Wv, rel_pos_emb, mask, **_unused):
    global LAST_EXEC_NS
    F16N = ml_dtypes.float16 if hasattr(ml_dtypes, "float16") else np.float16
    x = np.asarray(x, dtype=np.float32)
    Wk = np.asarray(Wk, dtype=np.float32)
    bk = np.asarray(bk, dtype=np.float32)
    Wq = np.asarray(Wq, dtype=np.float32)
    bq = np.asarray(bq, dtype=np.float32)
    Wv = np.asarray(Wv, dtype=np.float32)
    rel = np.asarray(rel_pos_emb, dtype=np.float32)
    causal = bool(np.asarray(mask).item())
    cfg = _cfg(causal)
    exts = cfg["exts"]
    NCHS = [e // 512 for e in exts]
    NGRP = 16

    scale = np.float32(np.sqrt(H))
    # x stream: [D,B,T] -> [32, 128, 8, 512] (chunk, p, n, col), col=(b,t)
    xT = np.ascontiguousarray(x.transpose(2, 0, 1)).astype(np.float16)
    xflat = xT.reshape(8, 128, B * T)
    xs = np.ascontiguousarray(
        xflat.reshape(8, 128, B * T // 512, 512).transpose(2, 1, 0, 3))

    wkv = np.empty((1024, 128), np.float32)
    wkv[:, 0:64] = Wk * scale
    wkv[:, 64:128] = Wv
    wkv = np.ascontiguousarray(
        wkv.reshape(8, 128, 128).transpose(1, 0, 2)).astype(np.float16)
    wq = np.ascontiguousarray(
        Wq.reshape(8, 128, 64).transpose(1, 0, 2)).astype(np.float16)
    bk8 = (bk * scale).reshape(H, 1).astype(np.float32)
    bqr = bq.reshape(H, 1).astype(np.float32)
    identb = np.eye(128, dtype=np.float16)

    # relT: [T, H, T] (t, c, v) in fp16
    relT = np.ascontiguousarray(rel.transpose(0, 2, 1)).astype(np.float16)

    def pack_rel_block(blkid, ext):
        # [TB, H, ext] -> [NGRP*nch, 128, 4, 512]
        nch = ext // 512
        a = relT[blkid * TB:(blkid + 1) * TB, :, 0:ext]
        a = a.reshape(NGRP, 4, 2, H, nch, 512)        # t=(g,u,j), c, (ch,v)
        a = a.transpose(0, 4, 2, 3, 1, 5)             # g, ch, j, c, u, v
        return a.reshape(NGRP * nch, 128, 4, 512)

    in_maps = []
    blocks = []
    for c in range(NCORES):
        bA, bB = c, NBLK - 1 - c
        blocks.append((bA, bB))
        rel_c = np.ascontiguousarray(np.concatenate(
            [pack_rel_block(bA, exts[0]), pack_rel_block(bB, exts[1])]))
        # xq stream: own columns, cols=(blk, b, t)
        xq = np.empty((8, 128, 2, B, TB), np.float16)
        xq[:, :, 0] = xflat.reshape(8, 128, B, T)[:, :, :, bA * TB:(bA + 1) * TB]
        xq[:, :, 1] = xflat.reshape(8, 128, B, T)[:, :, :, bB * TB:(bB + 1) * TB]
        xqs = np.ascontiguousarray(
            xq.reshape(8, 128, 2 * B * TB // 512, 512).transpose(2, 1, 0, 3))
        masks = []
        for slot, blkid in ((0, bA), (1, bB)):
            ext = exts[slot]
            t_idx = blkid * TB + np.arange(TB)[:, None]
            s_idx = np.arange(ext)[None, :]
            if causal:
                m = np.where(s_idx <= t_idx, 0.0, NEG)
            else:
                m = np.zeros((TB, ext))
            masks.append(np.ascontiguousarray(m.astype(ml_dtypes.bfloat16)))
        in_maps.append({
            "xs": xs, "xqs": xqs,
            "wkv": wkv, "wq": wq, "bk8": bk8, "bq": bqr,
            "rels": rel_c,
            "maskA": masks[0], "maskB": masks[1],
            "identb": identb,
        })

    nc = build_nc(cfg)
    if os.environ.get("KERNEL_TRACE") == "1":
        import jax
        jax.devices()
        try:
            res = run_bass_kernel_spmd(
                nc, in_maps, core_ids=list(range(NCORES)), trace=True)
        except (RuntimeError, ModuleNotFoundError):
            res = run_bass_kernel_spmd(
                nc, in_maps, core_ids=list(range(NCORES)))
    else:
        res = run_bass_kernel_spmd(nc, in_maps, core_ids=list(range(NCORES)))
    LAST_EXEC_NS = res.exec_time_ns

    out = np.empty((B, T, H), dtype=np.float32)
    for c in range(NCORES):
        oc = res.results[c]["out"]          # [B, 2, TB, H]
        bA, bB = blocks[c]
        out[:, bA * TB:(bA + 1) * TB] = oc[:, 0]
        out[:, bB * TB:(bB + 1) * TB] = oc[:, 1]
    return out


# revision 24
# speedup vs baseline: 2.7403x; 1.3964x over previous
"""Trainium2 Bass kernel for nn_AttentionHead (B=8, T=2048, D=1024, H=64).

Single attention head with additive relative-position scores:
    k = x@Wk + bk; q = x@Wq + bq; v = x@Wv
    S = (q k^T) sqrt(H) + einsum(btc,tvc->btv)(q, rel)  [+ causal mask]
    out = softmax(S) @ v

Distribution: query-block parallel over 8 NeuronCores. Core c owns query
blocks {c, 15-c} (128 rows each) so causal work is balanced. One SPMD
program runs on every core; per-core differences (which rel rows, which
causal mask, which q columns) are carried entirely by the input data.

Numerics (validated against the reference data on CPU): x, W and rel are
single-stream fp16 (exact products, fp32 accumulate on the PE). k and q
are computed in fp32 and split hi/lo bf16 on device for the q·k product
(error ~2^-17); the rel product runs single fp16 (error ~1e-3 on logits).
Measured end-to-end rel-err ~1.2e-2 vs the 2e-2 gate.
"""

import os
from contextlib import ExitStack

import numpy as np
import ml_dtypes

import concourse.bass as bass
import concourse.tile as tile
from concourse import bacc, mybir
from concourse.bass_utils import run_bass_kernel_spmd

BF16 = mybir.dt.bfloat16
F16 = mybir.dt.float16
F32 = mybir.dt.float32

# problem shape (hardcoded per contract)
B, T, D, H = 8, 2048, 1024, 64
TB = 128              # query-block rows
NBLK = T // TB        # 16
NCORES = 8
NEG = -1.0e9

LAST_EXEC_NS = None


def _cfg(causal: bool):
    # per-core uniform padded extents for the (small, big) block slots
    if causal:
        exts = (1024, 2048)
    else:
        exts = (2048, 2048)
    return {
        "B": B, "T": T, "D": D, "H": H, "TB": TB,
        "exts": exts, "smax": T,
    }


def build_nc(cfg):
    Bc, Tc, Dc, Hc, TBc = cfg["B"], cfg["T"], cfg["D"], cfg["H"], cfg["TB"]
    exts = cfg["exts"]
    smax = cfg["smax"]
    ND = Dc // 128                 # d-tiles
    SCH = 512                      # s-chunk for projections / scores
    NPAIR = TBc // 2               # 64 t-pairs per block
    NGRP = NPAIR // 4              # 16 groups of 4 pairs
    NKV = Bc * smax // SCH         # 32 kv-projection chunks
    NQC = 2 * Bc * TBc // SCH      # 4 q-projection chunks
    NCHS = [e // SCH for e in exts]
    NRELC = NGRP * sum(NCHS)       # rel stream chunks total

    nc = bacc.Bacc("TRN2", target_bir_lowering=False, debug=False,
                   num_devices=NCORES)

    # ---- I/O (all host-packed in exact DMA stream order) ----
    xs = nc.dram_tensor("xs", [NKV, 128, ND, SCH], F16, kind="ExternalInput")
    xqs = nc.dram_tensor("xqs", [NQC, 128, ND, SCH], F16, kind="ExternalInput")
    wkv = nc.dram_tensor("wkv", [128, ND, 128], F16, kind="ExternalInput")
    wq_ = nc.dram_tensor("wq", [128, ND, Hc], F16, kind="ExternalInput")
    bk8 = nc.dram_tensor("bk8", [Hc, 1], F32, kind="ExternalInput")
    bq_ = nc.dram_tensor("bq", [Hc, 1], F32, kind="ExternalInput")
    relsA = nc.dram_tensor("relsA", [NGRP, 128, NCHS[0], 4, SCH], F16,
                           kind="ExternalInput")
    relsB = nc.dram_tensor("relsB", [NGRP, 128, NCHS[1], 4, SCH], F16,
                           kind="ExternalInput")
    maskA = nc.dram_tensor("maskA", [TBc, exts[0]], BF16, kind="ExternalInput")
    maskB = nc.dram_tensor("maskB", [TBc, exts[1]], BF16, kind="ExternalInput")
    identb = nc.dram_tensor("identb", [128, 128], F16, kind="ExternalInput")
    out = nc.dram_tensor("out", [Bc, 2, TBc, Hc], F32, kind="ExternalOutput")

    NST = smax // 128              # s-tiles for V
    with tile.TileContext(nc) as tc:
        with (
            tc.tile_pool(name="persist", bufs=1) as pp,
            tc.tile_pool(name="weights", bufs=1) as pw,
            tc.tile_pool(name="S16pool", bufs=2) as pS,
            tc.tile_pool(name="sdram", bufs=1, space="DRAM") as pd,
        ):
            # k stacked: rows 0-63 = kT_hi, rows 64-127 = kT_lo ; cols (b, s)
            kstack = pp.tile([128, Bc * smax], BF16, tag="kstack")
            # q stacks: cols (blk, b, t)
            qmain = pp.tile([128, 2 * Bc * TBc], BF16, tag="qmain")
            qcorr = pp.tile([128, 2 * Bc * TBc], BF16, tag="qcorr")
            qf16 = pp.tile([Hc, 2 * Bc * TBc], F16, tag="qf16")
            # V natural: [s-part, (b, stile, h)]
            vnat = pp.tile([128, Bc * NST * Hc], F16, tag="vnat")
            mA = pp.tile([TBc, exts[0]], BF16, tag="maskA")
            mB = pp.tile([TBc, exts[1]], BF16, tag="maskB")
            idb = pw.tile([128, 128], F16, tag="identb")
            wkv_t = pw.tile([128, ND, 128], F16, tag="wkv")
            wq_t = pw.tile([128, ND, Hc], F16, tag="wq")
            bk_t = pw.tile([Hc, 1], F32, tag="bk")
            bq_t = pw.tile([Hc, 1], F32, tag="bq")

            nc.sync.dma_start(mA, maskA.ap())
            nc.sync.dma_start(mB, maskB.ap())
            nc.sync.dma_start(idb, identb.ap())
            nc.sync.dma_start(wkv_t, wkv.ap())
            nc.sync.dma_start(wq_t, wq_.ap())
            nc.sync.dma_start(bk_t, bk8.ap())
            nc.sync.dma_start(bq_t, bq_.ap())

            # DRAM bounce for the rel-score shuffle: layout [g, u, jslot, b, v]
            # matches the psr/stg partition order (p = 32u + 8j + b) exactly,
            # so the group write is one flat DMA and the per-batch tail read
            # gathers rows (g, u, j) -> t = 8g + 2u + j contiguously.
            sdram = [pd.tile([NGRP, 4, 4, Bc, exts[blk]], F16,
                             tag=f"sd{blk}", name=f"sd{blk}")
                     for blk in range(2)]

            with ExitStack() as stk:
                ent = stk.enter_context
                px = ent(tc.tile_pool(name="xstream", bufs=2))
                pst = ent(tc.tile_pool(name="pstage", bufs=2))
                pbd = ent(tc.tile_pool(name="bd", bufs=2))
                prel = ent(tc.tile_pool(name="relstream", bufs=2))
                pstage = ent(tc.tile_pool(name="stage", bufs=2))
                pSf = ent(tc.tile_pool(name="Sfull", bufs=2))
                pP = ent(tc.tile_pool(name="Ppool", bufs=2))
                pPT = ent(tc.tile_pool(name="PTpool", bufs=2))
                po = ent(tc.tile_pool(name="outpool", bufs=2))
                pstat = ent(tc.tile_pool(name="stats", bufs=4))
                ppmm = ent(tc.tile_pool(name="psmm512", bufs=2, space="PSUM"))
                ppr = ent(tc.tile_pool(name="psrel", bufs=2, space="PSUM"))
                ppt = ent(tc.tile_pool(name="pstrans", bufs=2, space="PSUM"))
                ppo = ent(tc.tile_pool(name="psout", bufs=2, space="PSUM"))

                # ---- q projection over own columns: cols (blk, b, t) ----
                for ci in range(NQC):
                    c0 = ci * SCH
                    xqt = px.tile([128, ND, SCH], F16, tag="xs")
                    nc.sync.dma_start(xqt, xqs.ap()[ci])
                    psq = ppmm.tile([128, SCH], F32, tag="mm512")
                    for dt_ in range(ND):
                        nc.tensor.matmul(psq[0:Hc, :], wq_t[:, dt_],
                                         xqt[:, dt_],
                                         start=(dt_ == 0), stop=(dt_ == ND - 1))
                    qtmp = pst.tile([Hc, SCH], F32, tag="ktmp")
                    nc.scalar.activation(qtmp, psq[0:Hc, :],
                                         mybir.ActivationFunctionType.Identity,
                                         bias=bq_t[:, :], scale=1.0)
                    cols = slice(c0, c0 + SCH)
                    nc.vector.tensor_copy(qf16[:, cols], qtmp)
                    nc.vector.tensor_copy(qmain[0:Hc, cols], qtmp)
                    nc.vector.tensor_tensor(
                        qmain[Hc:128, cols], qtmp,
                        qmain[0:Hc, cols], mybir.AluOpType.subtract)
                    nc.vector.tensor_copy(qcorr[Hc:128, cols],
                                          qmain[0:Hc, cols])
                    nc.vector.tensor_copy(qcorr[0:Hc, cols],
                                          qmain[Hc:128, cols])

                # ---- block-diagonal fp16 q tiles for both blocks ----
                bds = []
                for blk in range(2):
                    bd = pbd.tile([128, NPAIR * 16], F16, tag="bd",
                                  name=f"bd_{blk}")
                    nc.vector.memset(bd, 0.0)
                    qblk = (qf16[:, blk * Bc * TBc:(blk + 1) * Bc * TBc]
                            .rearrange("c (b t) -> c b t", b=Bc))
                    for j in range(2):
                        dst = (bd[j * Hc:(j + 1) * Hc]
                               .rearrange("c (p s) -> c p s", s=16)
                               [:, :, j * 8:j * 8 + 8])
                        src = qblk[:, :, j::2].rearrange("c b p -> c p b")
                        nc.vector.tensor_copy(dst, src)
                    bds.append(bd)

                # ---- generator: fused k/v projection chunks ----
                def kv_steps():
                    for ci in range(NKV):
                        c0 = ci * SCH
                        bidx = c0 // smax
                        s0 = c0 % smax
                        xt = px.tile([128, ND, SCH], F16, tag="xs")
                        nc.sync.dma_start(xt, xs.ap()[ci])
                        pskv = ppmm.tile([128, SCH], F32, tag="mm512")
                        for dt_ in range(ND):
                            nc.tensor.matmul(pskv, wkv_t[:, dt_], xt[:, dt_],
                                             start=(dt_ == 0),
                                             stop=(dt_ == ND - 1))
                        ktmp = pst.tile([Hc, SCH], F32, tag="ktmp")
                        nc.scalar.activation(
                            ktmp, pskv[0:Hc, :],
                            mybir.ActivationFunctionType.Identity,
                            bias=bk_t[:, :], scale=1.0)
                        cols = slice(c0, c0 + SCH)
                        nc.vector.tensor_copy(kstack[0:Hc, cols], ktmp)
                        nc.vector.tensor_tensor(
                            kstack[Hc:128, cols], ktmp,
                            kstack[0:Hc, cols], mybir.AluOpType.subtract)
                        # evacuate the v half on ACT as well: a DVE read of
                        # the same PSUM bank can run concurrently with the
                        # ACT read above — same-bank dual access is fatal.
                        vtmp = pst.tile([Hc, SCH], F16, tag="vtmp")
                        nc.scalar.activation(
                            vtmp, pskv[Hc:128, :],
                            mybir.ActivationFunctionType.Identity, scale=1.0)
                        for sub in range(SCH // 128):
                            pvt = ppt.tile([128, 128], F16, tag="ptrans")
                            nc.tensor.transpose(
                                pvt[:, 0:Hc], vtmp[:, sub * 128:(sub + 1) * 128],
                                idb[0:Hc, 0:Hc])
                            st = (s0 + sub * 128) // 128
                            nc.vector.tensor_copy(
                                vnat[:, bidx * NST * Hc + st * Hc:
                                     bidx * NST * Hc + (st + 1) * Hc],
                                pvt[:, 0:Hc])
                        yield

                # ---- generator: rel-score streaming for one block ----
                def rel_steps(blk):
                    ext = exts[blk]
                    nch = ext // SCH
                    bd = bds[blk]
                    rel_t = relsA if blk == 0 else relsB
                    for g in range(NGRP):
                        stg = pstage.tile([128, exts[1]], F16, tag="stage")
                        rht = prel.tile([128, nch, 4, SCH], F16, tag="rh")
                        nc.sync.dma_start(rht, rel_t.ap()[g])
                        for ch in range(nch):
                            v0 = ch * SCH
                            psr = ppr.tile([128, SCH], F32, tag="pr")
                            for u in range(4):
                                p = 4 * g + u
                                nc.tensor.matmul(
                                    psr[32 * u:32 * u + 16, :],
                                    bd[:, p * 16:p * 16 + 16], rht[:, ch, u],
                                    start=True, stop=True,
                                    tile_position=(0, 32 * u))
                            nc.vector.tensor_copy(stg[:, v0:v0 + SCH], psr)
                            yield
                        nc.scalar.dma_start(sdram[blk][g], stg[:, 0:ext])

                # ---- generator: per-(block,batch) tail ----
                def tail_steps(blk):
                    ext = exts[blk]
                    nch = ext // SCH
                    msk = mA if blk == 0 else mB
                    for b in range(Bc):
                        S = pSf.tile([TBc, exts[1]], F32, tag="Sfull")
                        S16t = pS.tile([TBc, exts[1]], F16, tag="S16t")
                        nc.scalar.dma_start(S16t[:, 0:ext],
                                            sdram[blk][:, :, 0:2, b, :])
                        qm = qmain[0:Hc, (blk * Bc + b) * TBc:
                                   (blk * Bc + b + 1) * TBc]
                        qc = qcorr[:, (blk * Bc + b) * TBc:
                                   (blk * Bc + b + 1) * TBc]
                        for ch in range(nch):
                            s0 = ch * SCH
                            psS = ppmm.tile([128, SCH], F32, tag="mm512")
                            cols = slice(b * smax + s0, b * smax + s0 + SCH)
                            nc.tensor.matmul(psS[0:TBc, :], qm,
                                             kstack[0:Hc, cols],
                                             start=True, stop=False)
                            nc.tensor.matmul(psS[0:TBc, :], qc, kstack[:, cols],
                                             start=False, stop=True)
                            nc.vector.tensor_tensor(
                                S[:, s0:s0 + SCH], psS[0:TBc, :],
                                S16t[:, s0:s0 + SCH],
                                mybir.AluOpType.add)
                            nc.gpsimd.tensor_tensor(
                                S[:, s0:s0 + SCH], S[:, s0:s0 + SCH],
                                msk[:, s0:s0 + SCH], mybir.AluOpType.add)
                        negmax = pstat.tile([TBc, 1], F32, tag="negmax")
                        zsum = pstat.tile([TBc, 1], F32, tag="zsum")
                        rz = pstat.tile([TBc, 1], F32, tag="rz")
                        nc.vector.tensor_reduce(negmax, S[:, 0:ext],
                                                mybir.AxisListType.X,
                                                mybir.AluOpType.max,
                                                negate=True)
                        P = pP.tile([TBc, exts[1]], F16, tag="P")
                        nc.scalar.activation(P[:, 0:ext], S[:, 0:ext],
                                             mybir.ActivationFunctionType.Exp,
                                             bias=negmax[:, :], scale=1.0,
                                             accum_out=zsum[:, :])
                        nc.vector.reciprocal(rz, zsum)
                        pso = ppo.tile([TBc, Hc], F32, tag="pso")
                        for st in range(ext // 128):
                            ptp = ppt.tile([128, 128], F16, tag="ptrans")
                            nc.tensor.transpose(
                                ptp, P[:, st * 128:(st + 1) * 128], idb)
                            ptt = pPT.tile([128, 128], F16, tag="ptt")
                            nc.vector.tensor_copy(ptt, ptp)
                            nc.tensor.matmul(
                                pso, ptt,
                                vnat[:, (b * NST + st) * Hc:
                                     (b * NST + st + 1) * Hc],
                                start=(st == 0), stop=(st == ext // 128 - 1))
                        osb = po.tile([TBc, Hc], F32, tag="osb")
                        nc.vector.tensor_scalar_mul(osb, pso, rz[:, :])
                        nc.sync.dma_start(out.ap()[b, blk], osb)
                        yield

                # ---- drive the streams ----
                # phase A: kv chunks with rel blk1 (the big block), 1 kv : 2 rel
                kv = kv_steps()
                r1 = rel_steps(1)
                done_kv = done_r1 = False
                while not (done_kv and done_r1):
                    if not done_kv:
                        done_kv = next(kv, "end") == "end"
                    for _ in range(2):
                        if not done_r1:
                            done_r1 = next(r1, "end") == "end"
                # phase B: rel blk0 interleaved with blk1 tails
                r0 = rel_steps(0)
                t1 = tail_steps(1)
                done_r0 = done_t1 = False
                while not (done_r0 and done_t1):
                    for _ in range(4):
                        if not done_r0:
                            done_r0 = next(r0, "end") == "end"
                    if not done_t1:
                        done_t1 = next(t1, "end") == "end"
                # phase C: blk0 tails
                for _ in tail_steps(0):
                    pass

    nc.compile()
    return nc


def kernel(x, Wk, bk, Wq, bq, Wv, rel_pos_emb, mask, **_unused):
    global LAST_EXEC_NS
    F16N = ml_dtypes.float16 if hasattr(ml_dtypes, "float16") else np.float16
    x = np.asarray(x, dtype=np.float32)
    Wk = np.asarray(Wk, dtype=np.float32)
    bk = np.asarray(bk, dtype=np.float32)
    Wq = np.asarray(Wq, dtype=np.float32)
    bq = np.asarray(bq, dtype=np.float32)
    Wv = np.asarray(Wv, dtype=np.float32)
    rel = np.asarray(rel_pos_emb, dtype=np.float32)
    causal = bool(np.asarray(mask).item())
    cfg = _cfg(causal)
    exts = cfg["exts"]
    NCHS = [e // 512 for e in exts]
    NGRP = 16

    scale = np.float32(np.sqrt(H))
    # x stream: [D,B,T] -> [32, 128, 8, 512] (chunk, p, n, col), col=(b,t)
    xT = np.ascontiguousarray(x.transpose(2, 0, 1)).astype(np.float16)
    xflat = xT.reshape(8, 128, B * T)
    xs = np.ascontiguousarray(
        xflat.reshape(8, 128, B * T // 512, 512).transpose(2, 1, 0, 3))

    wkv = np.empty((1024, 128), np.float32)
    wkv[:, 0:64] = Wk * scale
    wkv[:, 64:128] = Wv
    wkv = np.ascontiguousarray(
        wkv.reshape(8, 128, 128).transpose(1, 0, 2)).astype(np.float16)
    wq = np.ascontiguousarray(
        Wq.reshape(8, 128, 64).transpose(1, 0, 2)).astype(np.float16)
    bk8 = (bk * scale).reshape(H, 1).astype(np.float32)
    bqr = bq.reshape(H, 1).astype(np.float32)
    identb = np.eye(128, dtype=np.float16)

    # relT: [T, H, T] (t, c, v) in fp16
    relT = np.ascontiguousarray(rel.transpose(0, 2, 1)).astype(np.float16)

    def pack_rel_block(blkid, ext):
        # [TB, H, ext] -> [NGRP, 128, nch, 4, 512]
        nch = ext // 512
        a = relT[blkid * TB:(blkid + 1) * TB, :, 0:ext]
        a = a.reshape(NGRP, 4, 2, H, nch, 512)        # t=(g,u,j), c, (ch,v)
        a = a.transpose(0, 2, 3, 4, 1, 5)             # g, j, c, ch, u, v
        return np.ascontiguousarray(
            a.reshape(NGRP, 128, nch, 4, 512))

    in_maps = []
    blocks = []
    for c in range(NCORES):
        bA, bB = c, NBLK - 1 - c
        blocks.append((bA, bB))
        relsA_c = pack_rel_block(bA, exts[0])
        relsB_c = pack_rel_block(bB, exts[1])
        # xq stream: own columns, cols=(blk, b, t)
        xq = np.empty((8, 128, 2, B, TB), np.float16)
        xq[:, :, 0] = xflat.reshape(8, 128, B, T)[:, :, :, bA * TB:(bA + 1) * TB]
        xq[:, :, 1] = xflat.reshape(8, 128, B, T)[:, :, :, bB * TB:(bB + 1) * TB]
        xqs = np.ascontiguousarray(
            xq.reshape(8, 128, 2 * B * TB // 512, 512).transpose(2, 1, 0, 3))
        masks = []
        for slot, blkid in ((0, bA), (1, bB)):
            ext = exts[slot]
            t_idx = blkid * TB + np.arange(TB)[:, None]
            s_idx = np.arange(ext)[None, :]
            if causal:
                m = np.where(s_idx <= t_idx, 0.0, NEG)
            else:
                m = np.zeros((TB, ext))
            masks.append(np.ascontiguousarray(m.astype(ml_dtypes.bfloat16)))
        in_maps.append({
            "xs": xs, "xqs": xqs,
            "wkv": wkv, "wq": wq, "bk8": bk8, "bq": bqr,
            "relsA": relsA_c, "relsB": relsB_c,
            "maskA": masks[0], "maskB": masks[1],
            "identb": identb,
        })

    nc = build_nc(cfg)
    if os.environ.get("KERNEL_TRACE") == "1":
        import jax
        jax.devices()
        try:
            res = run_bass_kernel_spmd(
                nc, in_maps, core_ids=list(range(NCORES)), trace=True)
        except (RuntimeError, ModuleNotFoundError):
            res = run_bass_kernel_spmd(
                nc, in_maps, core_ids=list(range(NCORES)))
    else:
        res = run_bass_kernel_spmd(nc, in_maps, core_ids=list(range(NCORES)))
    LAST_EXEC_NS = res.exec_time_ns

    out = np.empty((B, T, H), dtype=np.float32)
    for c in range(NCORES):
        oc = res.results[c]["out"]          # [B, 2, TB, H]
        bA, bB = blocks[c]
        out[:, bA * TB:(bA + 1) * TB] = oc[:, 0]
        out[:, bB * TB:(bB + 1) * TB] = oc[:, 1]
    return out
